# revision 12
# baseline (speedup 1.0000x reference)
"""Trainium2 Bass kernel for CusMultiHeadAttention (v2).

Shapes (hardcoded): x (4,1024,1024) f32, bias (4,16,1024,1024) f32,
attention_mask (4,1024) i32, Wq/Wk/Wv (1024,1024), Wo (1024,1024), bo (1024,).

Sharding: 8 cores = 4 batches x 2 head-groups (8 heads each).
Wq/Wk/Wv column-parallel, Wo row-parallel (host sums the pair partials + bo).

Key ideas vs v1:
  * mask-permutation: softmax sums are permutation-invariant over k, so the
    host reorders k-positions unmasked-first and the device only processes
    ku = ceil(max_unmasked/128) k-tiles (~5 of 8). Padded/masked positions
    get eb = 0, contributing exactly 0 (matches the reference's -1e8 offset).
  * host-side eb = exp(bias)*mask: the on-device bias add disappears;
    pt = exp(scores) * eb runs on DVE in 2x fp16 mode.
  * fp16 everywhere (same PE speed as bf16, 8x finer mantissa).
  * output projection restructured as outT = Wo'^T @ o^T with weights
    stationary and head pairs stacked on 128 partitions (halves its rows).
  * projections interleaved with attention head pairs to keep the PE at
    max p-state and every engine busy.
"""

import sys

if "/opt/trn_rl_repo" not in sys.path:
    sys.path.insert(0, "/opt/trn_rl_repo")

import math
from contextlib import ExitStack

import numpy as np

import concourse.mybir as mybir
import concourse.tile as tile
from concourse import bacc
from concourse.alu_op_type import AluOpType
from concourse.bass_utils import run_bass_kernel_spmd

B, S, C_IN = 4, 1024, 1024
N_HEAD, C = 16, 64
N_CORES = 8
HG = 8  # heads per core
F = HG * C  # 512 local features
P = 128
KT = C_IN // P  # 8 contraction tiles for projections
VW = C + 1  # 65: v columns + ones-column
NG = HG // 2  # 4 head pairs

f32 = mybir.dt.float32
f16 = mybir.dt.float16


def build_program(ku):
    KB = ku * P  # k positions kept
    nc = bacc.Bacc("TRN2", target_bir_lowering=False, debug=False,
                   num_devices=N_CORES)

    xq = nc.dram_tensor("xq", (C_IN, S), f16, kind="ExternalInput").ap()
    xk = nc.dram_tensor("xk", (C_IN, KB), f16, kind="ExternalInput").ap()
    wq = nc.dram_tensor("wq", (C_IN, F), f16, kind="ExternalInput").ap()
    wk = nc.dram_tensor("wk", (C_IN, F), f16, kind="ExternalInput").ap()
    wv = nc.dram_tensor("wv", (C_IN, F), f16, kind="ExternalInput").ap()
    wo2 = nc.dram_tensor("wo2", (P, NG, C_IN), f16, kind="ExternalInput").ap()
    ebT = nc.dram_tensor("ebT", (HG, KB, S), f16, kind="ExternalInput").ap()
    outT = nc.dram_tensor("outT", (C_IN, S), f16, kind="ExternalOutput").ap()

    with tile.TileContext(nc) as tc:
        with ExitStack() as ctx:
            persist = ctx.enter_context(tc.tile_pool(name="persist", bufs=1))
            v_sb = persist.tile([P, ku, HG * VW], f16)
            qT_sb = persist.tile([P, NG, S], f16)
            kT_sb = persist.tile([P, NG, KB], f16)
            wo2_sb = persist.tile([P, NG, C_IN], f16)
            oT2_sb = persist.tile([P, NG, S], f16)
            nc.sync.dma_start(wo2_sb[:], wo2)
            # ones columns for the softmax denominator (k-padding is killed
            # by eb == 0, so the ones column itself is unmasked)
            v_view = v_sb.rearrange("p t (h c) -> p t h c", c=VW)
            nc.vector.memset(v_view[:, :, :, C:C + 1], 1.0)

            # ---- phase A input staging ----
            pa = ctx.enter_context(tc.tile_pool(name="phaseA", bufs=1))
            xq_sb = pa.tile([P, KT, S], f16)
            xk_sb = pa.tile([P, KT, KB], f16)
            wq_sb = pa.tile([P, KT, F], f16)
            wk_sb = pa.tile([P, KT, F], f16)
            wv_sb = pa.tile([P, KT, F], f16)
            # issue order matters: qkproj(0) consumes wq/xq (then wk/xk)
            # first, so those transfers go out ahead of wv.
            for kt in range(KT):
                sl = slice(kt * P, (kt + 1) * P)
                nc.sync.dma_start(wq_sb[:, kt, :], wq[sl, :])
                nc.sync.dma_start(xq_sb[:, kt, :], xq[sl, :])
            for kt in range(KT):
                sl = slice(kt * P, (kt + 1) * P)
                nc.sync.dma_start(wk_sb[:, kt, :], wk[sl, :])
                nc.sync.dma_start(xk_sb[:, kt, :], xk[sl, :])
            for kt in range(KT):
                sl = slice(kt * P, (kt + 1) * P)
                nc.sync.dma_start(wv_sb[:, kt, :], wv[sl, :])

            # ---- interleaved q/k projections + attention ----
            attn_ctx = ctx.enter_context(ExitStack())
            ps_pool = attn_ctx.enter_context(
                tc.tile_pool(name="ps", bufs=3, space="PSUM"))
            oap_pool = attn_ctx.enter_context(
                tc.tile_pool(name="oap", bufs=2, space="PSUM"))
            ebt_pool = attn_ctx.enter_context(tc.tile_pool(name="ebt", bufs=3))
            ptr_pool = attn_ctx.enter_context(tc.tile_pool(name="ptr", bufs=3))
            pt_pool = attn_ctx.enter_context(tc.tile_pool(name="pt", bufs=3))
            rc_pool = attn_ctx.enter_context(tc.tile_pool(name="rc", bufs=2))
            rcb_pool = attn_ctx.enter_context(tc.tile_pool(name="rcb", bufs=2))
            tmpo_pool = attn_ctx.enter_context(
                tc.tile_pool(name="tmpo", bufs=2))

            def qkproj(mt):
                for w_sb, dst, nfree in ((wq_sb, qT_sb, S), (wk_sb, kT_sb, KB)):
                    for lo in range(0, nfree, 512):
                        hi = min(lo + 512, nfree)
                        ps = ps_pool.tile([P, 512], f32, name="ps")
                        for kt in range(KT):
                            nc.tensor.matmul(
                                ps[:, 0:hi - lo],
                                w_sb[:, kt, mt * P:(mt + 1) * P],
                                (xq_sb if dst is qT_sb else xk_sb)[
                                    :, kt, lo:hi],
                                start=(kt == 0), stop=(kt == KT - 1))
                        nc.vector.tensor_copy(dst[:, mt, lo:hi],
                                              ps[:, 0:hi - lo])

            def vproj(mt):
                psv = ps_pool.tile([P, F], f32, name="ps")
                for kt in range(KT):
                    nc.tensor.matmul(
                        psv[:],
                        xk_sb[:, kt, mt * P:(mt + 1) * P],
                        wv_sb[:, kt, :],
                        start=(kt == 0), stop=(kt == KT - 1))
                nc.vector.tensor_copy(
                    v_view[:, mt, :, 0:C],
                    psv.rearrange("p (h c) -> p h c", c=C))

            def attn(h):
                g = h // 2
                po = (h % 2) * C
                kT_h = kT_sb[po:po + C, g, :]
                qT_h = qT_sb[po:po + C, g, :]
                oaps = oap_pool.tile([VW, S], f32, name="oaug")
                for kt in range(ku):
                    ebt = ebt_pool.tile([P, S], f16, name="ebt")
                    nc.sync.dma_start(ebt[:], ebT[h, kt * P:(kt + 1) * P, :])
                    for nh in range(2):
                        ps_s = ps_pool.tile([P, 512], f32, name="ps")
                        nc.tensor.matmul(
                            ps_s[:],
                            kT_h[:, kt * P:(kt + 1) * P],
                            qT_h[:, nh * 512:(nh + 1) * 512],
                            start=True, stop=True)
                        ptr = ptr_pool.tile([P, 512], f16, name="ptr")
                        nc.scalar.activation(ptr[:], ps_s[:],
                                             mybir.ActivationFunctionType.Exp)
                        pt = pt_pool.tile([P, 512], f16, name="pt")
                        # split the eb multiplies between DVE and Pool
                        eng = nc.gpsimd if (kt * 2 + nh) % 3 == 2 else nc.vector
                        eng.tensor_mul(pt[:], ptr[:],
                                       ebt[:, nh * 512:(nh + 1) * 512])
                        nc.tensor.matmul(
                            oaps[:, nh * 512:(nh + 1) * 512],
                            v_sb[:, kt, h * VW:(h + 1) * VW],
                            pt[:],
                            start=(kt == 0), stop=(kt == ku - 1))
                # denominator: row C of oaps -> partition 0, recip, broadcast
                rc = rc_pool.tile([P, S], f32, name="rc")
                nc.vector.tensor_copy(rc[C:C + 1, :], oaps[C:C + 1, :])
                rc0 = rc_pool.tile([1, S], f32, name="rc0", tag="rc0")
                nc.sync.dma_start(rc0[:], rc[C:C + 1, :])
                rcv = rc_pool.tile([1, S], f32, name="rcv", tag="rcv")
                nc.vector.reciprocal_approx_fast(rcv[:], rc0[:])
                rcb = rcb_pool.tile([C, S], f32, name="rcb")
                nc.gpsimd.partition_broadcast(rcb[:], rcv[:])
                if h % 2 == 0:
                    nc.vector.tensor_mul(oT2_sb[0:C, g, :], oaps[0:C, :],
                                         rcb[:])
                else:
                    tmp = tmpo_pool.tile([C, S], f16, name="tmpo")
                    nc.vector.tensor_mul(tmp[:], oaps[0:C, :], rcb[:])
                    nc.sync.dma_start(oT2_sb[C:P, g, :], tmp[:])

            qkproj(0)
            for mt in range(ku):
                vproj(mt)
            attn(0)
            qkproj(1)
            attn(1)
            qkproj(2)
            attn(2)
            attn(3)
            qkproj(3)
            for h in range(4, HG):
                attn(h)
            attn_ctx.close()

            # ---- output projection: outT = wo2^T @ oT2 (pair-stacked) ----
            with tc.tile_pool(name="psOut", bufs=2, space="PSUM") as psOut, \
                 tc.tile_pool(name="outsb", bufs=3) as out_pool:
                for ct in range(KT):
                    for nh in range(2):
                        pso = psOut.tile([P, 512], f32, name="pso")
                        for g in range(NG):
                            nc.tensor.matmul(
                                pso[:],
                                wo2_sb[:, g, ct * P:(ct + 1) * P],
                                oT2_sb[:, g, nh * 512:(nh + 1) * 512],
                                start=(g == 0), stop=(g == NG - 1))
                        osb = out_pool.tile([P, 512], f16, name="osb")
                        nc.scalar.copy(osb[:], pso[:])
                        nc.sync.dma_start(
                            outT[ct * P:(ct + 1) * P,
                                 nh * 512:(nh + 1) * 512],
                            osb[:])

    nc.compile()
    return nc


def make_in_maps(x, bias, attention_mask, Wq, Wk, Wv, Wo):
    x = np.asarray(x)
    bias = np.asarray(bias)
    mask = np.asarray(attention_mask)
    scale = 1.0 / math.sqrt(C)
    wq16 = (np.asarray(Wq) * scale).astype(np.float16)
    wk16 = np.asarray(Wk).astype(np.float16)
    wv16 = np.asarray(Wv).astype(np.float16)
    wo = np.asarray(Wo)

    counts = mask.sum(axis=1)
    ku = max(1, int(math.ceil(counts.max() / P)))
    KB = ku * P

    # per-batch permutation: unmasked k-positions first
    idxs = []
    for b in range(B):
        order = np.argsort(~mask[b].astype(bool), kind="stable")
        idxs.append(order[:KB])

    in_maps = []
    for c in range(N_CORES):
        b, hg = c // 2, c % 2
        fs = slice(hg * F, (hg + 1) * F)
        idx = idxs[b]
        xT = x[b].T.astype(np.float16)  # (c_in, S)
        # eb = exp(bias) * mask, permuted/truncated on k, transposed to (k,q)
        eb = np.exp(bias[b, hg * HG:(hg + 1) * HG][:, :, idx])
        eb *= mask[b][idx].astype(np.float32)[None, None, :]
        ebT = np.ascontiguousarray(eb.transpose(0, 2, 1)).astype(np.float16)
        wo_c = wo[fs].astype(np.float16)  # (512, c_in)
        wo2 = np.ascontiguousarray(
            wo_c.reshape(NG, 2, C, C_IN).transpose(1, 2, 0, 3)
        ).reshape(P, NG, C_IN)
        in_maps.append({
            "xq": np.ascontiguousarray(xT),
            "xk": np.ascontiguousarray(xT[:, idx]),
            "wq": np.ascontiguousarray(wq16[:, fs]),
            "wk": np.ascontiguousarray(wk16[:, fs]),
            "wv": np.ascontiguousarray(wv16[:, fs]),
            "wo2": wo2,
            "ebT": ebT,
        })
    return in_maps, ku


_NC_CACHE = {}


def get_program(ku=5):
    if ku not in _NC_CACHE:
        _NC_CACHE[ku] = build_program(ku)
    return _NC_CACHE[ku]


def run(in_maps, ku, trace=False, **kw):
    nc = get_program(ku)
    return run_bass_kernel_spmd(nc, in_maps, core_ids=list(range(N_CORES)),
                                trace=trace, **kw)


def kernel(x, bias, attention_mask, Wq, Wk, Wv, Wo, bo):
    in_maps, ku = make_in_maps(x, bias, attention_mask, Wq, Wk, Wv, Wo)
    res = run(in_maps, ku)
    out = np.empty((B, S, C_IN), dtype=np.float32)
    bo32 = np.asarray(bo).astype(np.float32)
    for b in range(B):
        acc = (res.results[2 * b]["outT"].astype(np.float32)
               + res.results[2 * b + 1]["outT"].astype(np.float32))
        out[b] = acc.T + bo32
    return out


# revision 18
# speedup vs baseline: 1.5000x; 1.5000x over previous
"""Trainium2 Bass kernel for CusMultiHeadAttention (v2).

Shapes (hardcoded): x (4,1024,1024) f32, bias (4,16,1024,1024) f32,
attention_mask (4,1024) i32, Wq/Wk/Wv (1024,1024), Wo (1024,1024), bo (1024,).

Sharding: 8 cores = 4 batches x 2 head-groups (8 heads each).
Wq/Wk/Wv column-parallel, Wo row-parallel (host sums the pair partials + bo).

Key ideas vs v1:
  * mask-permutation: softmax sums are permutation-invariant over k, so the
    host reorders k-positions unmasked-first and the device only processes
    ku = ceil(max_unmasked/128) k-tiles (~5 of 8). Padded/masked positions
    get eb = 0, contributing exactly 0 (matches the reference's -1e8 offset).
  * host-side eb = exp(bias)*mask: the on-device bias add disappears;
    pt = exp(scores) * eb runs on DVE in 2x fp16 mode.
  * fp16 everywhere (same PE speed as bf16, 8x finer mantissa).
  * output projection restructured as outT = Wo'^T @ o^T with weights
    stationary and head pairs stacked on 128 partitions (halves its rows).
  * projections interleaved with attention head pairs to keep the PE at
    max p-state and every engine busy.
"""

import sys

if "/opt/trn_rl_repo" not in sys.path:
    sys.path.insert(0, "/opt/trn_rl_repo")

import math
from contextlib import ExitStack

import numpy as np

import concourse.mybir as mybir
import concourse.tile as tile
from concourse import bacc
from concourse.alu_op_type import AluOpType
from concourse.bass_utils import run_bass_kernel_spmd

B, S, C_IN = 4, 1024, 1024
N_HEAD, C = 16, 64
N_CORES = 8
HG = 8  # heads per core
F = HG * C  # 512 local features
P = 128
KT = C_IN // P  # 8 contraction tiles for projections
VW = C + 1  # 65: v columns + ones-column
NG = HG // 2  # 4 head pairs

f32 = mybir.dt.float32
f16 = mybir.dt.float16


def build_program(ku):
    KB = ku * P  # k positions kept
    nc = bacc.Bacc("TRN2", target_bir_lowering=False, debug=False,
                   num_devices=N_CORES)

    xq = nc.dram_tensor("xq", (C_IN, S), f16, kind="ExternalInput").ap()
    xk = nc.dram_tensor("xk", (C_IN, KB), f16, kind="ExternalInput").ap()
    wq = nc.dram_tensor("wq", (C_IN, F), f16, kind="ExternalInput").ap()
    wk = nc.dram_tensor("wk", (C_IN, F), f16, kind="ExternalInput").ap()
    wv = nc.dram_tensor("wv", (C_IN, F), f16, kind="ExternalInput").ap()
    wo2 = nc.dram_tensor("wo2", (P, NG, C_IN), f16, kind="ExternalInput").ap()
    ebT = nc.dram_tensor("ebT", (HG, KB, S), f16, kind="ExternalInput").ap()
    outT = nc.dram_tensor("outT", (C_IN, S), f16, kind="ExternalOutput").ap()

    with tile.TileContext(nc) as tc:
        with ExitStack() as ctx:
            persist = ctx.enter_context(tc.tile_pool(name="persist", bufs=1))
            v_sb = persist.tile([P, ku, HG * VW], f16)
            qT_sb = persist.tile([P, NG, S], f16)
            kT_sb = persist.tile([P, NG, KB], f16)
            wo2_sb = persist.tile([P, NG, C_IN], f16)
            oT2_sb = persist.tile([P, NG, S], f16)
            nc.sync.dma_start(wo2_sb[:], wo2)
            # ones columns for the softmax denominator (k-padding is killed
            # by eb == 0, so the ones column itself is unmasked)
            v_view = v_sb.rearrange("p t (h c) -> p t h c", c=VW)
            nc.vector.memset(v_view[:, :, :, C:C + 1], 1.0)

            # ---- phase A input staging ----
            pa = ctx.enter_context(tc.tile_pool(name="phaseA", bufs=1))
            xq_sb = pa.tile([P, KT, S], f16)
            xk_sb = pa.tile([P, KT, KB], f16)
            wq_sb = pa.tile([P, KT, F], f16)
            wk_sb = pa.tile([P, KT, F], f16)
            wv_sb = pa.tile([P, KT, F], f16)
            # one batched DMA per tensor (per-tile DMAs pay ~0.6us of issue
            # cost each); issue order matters: qkproj(0) consumes wq/xq.
            nc.sync.dma_start(wq_sb[:], wq.rearrange("(t p) f -> p t f", p=P))
            nc.sync.dma_start(xq_sb[:], xq.rearrange("(t p) s -> p t s", p=P))
            nc.sync.dma_start(wk_sb[:], wk.rearrange("(t p) f -> p t f", p=P))
            nc.sync.dma_start(xk_sb[:], xk.rearrange("(t p) s -> p t s", p=P))
            nc.sync.dma_start(wv_sb[:], wv.rearrange("(t p) f -> p t f", p=P))

            # ---- interleaved q/k projections + attention ----
            attn_ctx = ctx.enter_context(ExitStack())
            ps_pool = attn_ctx.enter_context(
                tc.tile_pool(name="ps", bufs=3, space="PSUM"))
            oap_pool = attn_ctx.enter_context(
                tc.tile_pool(name="oap", bufs=2, space="PSUM"))
            ebt_pool = attn_ctx.enter_context(tc.tile_pool(name="ebt", bufs=3))
            ptr_pool = attn_ctx.enter_context(tc.tile_pool(name="ptr", bufs=3))
            pt_pool = attn_ctx.enter_context(tc.tile_pool(name="pt", bufs=3))
            rc_pool = attn_ctx.enter_context(tc.tile_pool(name="rc", bufs=2))
            rcb_pool = attn_ctx.enter_context(tc.tile_pool(name="rcb", bufs=2))
            tmpo_pool = attn_ctx.enter_context(
                tc.tile_pool(name="tmpo", bufs=2))

            def qkproj(mt):
                for w_sb, dst, nfree in ((wq_sb, qT_sb, S), (wk_sb, kT_sb, KB)):
                    for lo in range(0, nfree, 512):
                        hi = min(lo + 512, nfree)
                        ps = ps_pool.tile([P, 512], f32, name="ps")
                        for kt in range(KT):
                            nc.tensor.matmul(
                                ps[:, 0:hi - lo],
                                w_sb[:, kt, mt * P:(mt + 1) * P],
                                (xq_sb if dst is qT_sb else xk_sb)[
                                    :, kt, lo:hi],
                                start=(kt == 0), stop=(kt == KT - 1))
                        nc.scalar.copy(dst[:, mt, lo:hi], ps[:, 0:hi - lo])

            def vproj(mt):
                psv = ps_pool.tile([P, F], f32, name="ps")
                for kt in range(KT):
                    nc.tensor.matmul(
                        psv[:],
                        xk_sb[:, kt, mt * P:(mt + 1) * P],
                        wv_sb[:, kt, :],
                        start=(kt == 0), stop=(kt == KT - 1))
                nc.vector.tensor_copy(
                    v_view[:, mt, :, 0:C],
                    psv.rearrange("p (h c) -> p h c", c=C))

            def attn(h):
                g = h // 2
                po = (h % 2) * C
                kT_h = kT_sb[po:po + C, g, :]
                qT_h = qT_sb[po:po + C, g, :]
                oaps = oap_pool.tile([VW, S], f32, name="oaug")
                for kt in range(ku):
                    ebt = ebt_pool.tile([P, S], f16, name="ebt")
                    nc.sync.dma_start(ebt[:], ebT[h, kt * P:(kt + 1) * P, :])
                    for nh in range(2):
                        ps_s = ps_pool.tile([P, 512], f32, name="ps")
                        nc.tensor.matmul(
                            ps_s[:],
                            kT_h[:, kt * P:(kt + 1) * P],
                            qT_h[:, nh * 512:(nh + 1) * 512],
                            start=True, stop=True)
                        ptr = ptr_pool.tile([P, 512], f16, name="ptr")
                        nc.scalar.activation(ptr[:], ps_s[:],
                                             mybir.ActivationFunctionType.Exp)
                        pt = pt_pool.tile([P, 512], f16, name="pt")
                        nc.vector.tensor_mul(pt[:], ptr[:],
                                             ebt[:, nh * 512:(nh + 1) * 512])
                        nc.tensor.matmul(
                            oaps[:, nh * 512:(nh + 1) * 512],
                            v_sb[:, kt, h * VW:(h + 1) * VW],
                            pt[:],
                            start=(kt == 0), stop=(kt == ku - 1))
                # denominator row C of oaps -> partition 0 via SBUF DMA hop,
                # recip, gpsimd broadcast
                rc = rc_pool.tile([P, S], f32, name="rc")
                nc.vector.tensor_copy(rc[C:C + 1, :], oaps[C:C + 1, :])
                rc0 = rc_pool.tile([1, S], f32, name="rc0", tag="rc0")
                nc.sync.dma_start(rc0[:], rc[C:C + 1, :])
                rcv = rc_pool.tile([1, S], f32, name="rcv", tag="rcv")
                nc.vector.reciprocal_approx_fast(rcv[:], rc0[:])
                rcb = rcb_pool.tile([C, S], f32, name="rcb")
                nc.gpsimd.partition_broadcast(rcb[:], rcv[:])
                if h % 2 == 0:
                    nc.vector.tensor_mul(oT2_sb[0:C, g, :], oaps[0:C, :],
                                         rcb[:])
                else:
                    tmp = tmpo_pool.tile([C, S], f16, name="tmpo")
                    nc.vector.tensor_mul(tmp[:], oaps[0:C, :], rcb[:])
                    nc.sync.dma_start(oT2_sb[C:P, g, :], tmp[:])

            qkproj(0)
            for mt in range(ku):
                vproj(mt)
            attn(0)
            qkproj(1)
            attn(1)
            qkproj(2)
            attn(2)
            attn(3)
            qkproj(3)
            for h in range(4, HG):
                attn(h)
            attn_ctx.close()

            # ---- output projection: outT = wo2^T @ oT2 (pair-stacked) ----
            with tc.tile_pool(name="psOut", bufs=2, space="PSUM") as psOut, \
                 tc.tile_pool(name="outsb", bufs=3) as out_pool:
                for ct in range(KT):
                    for nh in range(2):
                        pso = psOut.tile([P, 512], f32, name="pso")
                        for g in range(NG):
                            nc.tensor.matmul(
                                pso[:],
                                wo2_sb[:, g, ct * P:(ct + 1) * P],
                                oT2_sb[:, g, nh * 512:(nh + 1) * 512],
                                start=(g == 0), stop=(g == NG - 1))
                        osb = out_pool.tile([P, 512], f16, name="osb")
                        nc.scalar.copy(osb[:], pso[:])
                        nc.sync.dma_start(
                            outT[ct * P:(ct + 1) * P,
                                 nh * 512:(nh + 1) * 512],
                            osb[:])

    nc.compile()
    return nc


def make_in_maps(x, bias, attention_mask, Wq, Wk, Wv, Wo):
    x = np.asarray(x)
    bias = np.asarray(bias)
    mask = np.asarray(attention_mask)
    scale = 1.0 / math.sqrt(C)
    wq16 = (np.asarray(Wq) * scale).astype(np.float16)
    wk16 = np.asarray(Wk).astype(np.float16)
    wv16 = np.asarray(Wv).astype(np.float16)
    wo = np.asarray(Wo)

    counts = mask.sum(axis=1)
    ku = max(1, int(math.ceil(counts.max() / P)))
    KB = ku * P

    # per-batch permutation: unmasked k-positions first
    idxs = []
    for b in range(B):
        order = np.argsort(~mask[b].astype(bool), kind="stable")
        idxs.append(order[:KB])

    in_maps = []
    for c in range(N_CORES):
        b, hg = c // 2, c % 2
        fs = slice(hg * F, (hg + 1) * F)
        idx = idxs[b]
        xT = x[b].T.astype(np.float16)  # (c_in, S)
        # eb = exp(bias) * mask, permuted/truncated on k, transposed to (k,q)
        eb = np.exp(bias[b, hg * HG:(hg + 1) * HG][:, :, idx])
        eb *= mask[b][idx].astype(np.float32)[None, None, :]
        ebT = np.ascontiguousarray(eb.transpose(0, 2, 1)).astype(np.float16)
        wo_c = wo[fs].astype(np.float16)  # (512, c_in)
        wo2 = np.ascontiguousarray(
            wo_c.reshape(NG, 2, C, C_IN).transpose(1, 2, 0, 3)
        ).reshape(P, NG, C_IN)
        in_maps.append({
            "xq": np.ascontiguousarray(xT),
            "xk": np.ascontiguousarray(xT[:, idx]),
            "wq": np.ascontiguousarray(wq16[:, fs]),
            "wk": np.ascontiguousarray(wk16[:, fs]),
            "wv": np.ascontiguousarray(wv16[:, fs]),
            "wo2": wo2,
            "ebT": ebT,
        })
    return in_maps, ku


_NC_CACHE = {}


def get_program(ku=5):
    if ku not in _NC_CACHE:
        _NC_CACHE[ku] = build_program(ku)
    return _NC_CACHE[ku]


def run(in_maps, ku, trace=False, **kw):
    nc = get_program(ku)
    return run_bass_kernel_spmd(nc, in_maps, core_ids=list(range(N_CORES)),
                                trace=trace, **kw)


def kernel(x, bias, attention_mask, Wq, Wk, Wv, Wo, bo):
    in_maps, ku = make_in_maps(x, bias, attention_mask, Wq, Wk, Wv, Wo)
    res = run(in_maps, ku)
    out = np.empty((B, S, C_IN), dtype=np.float32)
    bo32 = np.asarray(bo).astype(np.float32)
    for b in range(B):
        acc = (res.results[2 * b]["outT"].astype(np.float32)
               + res.results[2 * b + 1]["outT"].astype(np.float32))
        out[b] = acc.T + bo32
    return out


# revision 22
# speedup vs baseline: 1.5187x; 1.0124x over previous
"""Trainium2 Bass kernel for CusMultiHeadAttention (v2).

Shapes (hardcoded): x (4,1024,1024) f32, bias (4,16,1024,1024) f32,
attention_mask (4,1024) i32, Wq/Wk/Wv (1024,1024), Wo (1024,1024), bo (1024,).

Sharding: 8 cores = 4 batches x 2 head-groups (8 heads each).
Wq/Wk/Wv column-parallel, Wo row-parallel (host sums the pair partials + bo).

Key ideas vs v1:
  * mask-permutation: softmax sums are permutation-invariant over k, so the
    host reorders k-positions unmasked-first and the device only processes
    ku = ceil(max_unmasked/128) k-tiles (~5 of 8). Padded/masked positions
    get eb = 0, contributing exactly 0 (matches the reference's -1e8 offset).
  * host-side eb = exp(bias)*mask: the on-device bias add disappears;
    pt = exp(scores) * eb runs on DVE in 2x fp16 mode.
  * fp16 everywhere (same PE speed as bf16, 8x finer mantissa).
  * output projection restructured as outT = Wo'^T @ o^T with weights
    stationary and head pairs stacked on 128 partitions (halves its rows).
  * projections interleaved with attention head pairs to keep the PE at
    max p-state and every engine busy.
"""

import sys

if "/opt/trn_rl_repo" not in sys.path:
    sys.path.insert(0, "/opt/trn_rl_repo")

import math
from contextlib import ExitStack

import numpy as np

import concourse.mybir as mybir
import concourse.tile as tile
from concourse import bacc
from concourse.alu_op_type import AluOpType
from concourse.bass_utils import run_bass_kernel_spmd

B, S, C_IN = 4, 1024, 1024
N_HEAD, C = 16, 64
N_CORES = 8
HG = 8  # heads per core
F = HG * C  # 512 local features
P = 128
KT = C_IN // P  # 8 contraction tiles for projections
VW = C + 1  # 65: v columns + ones-column
NG = HG // 2  # 4 head pairs

f32 = mybir.dt.float32
f16 = mybir.dt.float16


def build_program(ku):
    KB = ku * P  # k positions kept
    nc = bacc.Bacc("TRN2", target_bir_lowering=False, debug=False,
                   num_devices=N_CORES)

    xq = nc.dram_tensor("xq", (C_IN, S), f16, kind="ExternalInput").ap()
    xk = nc.dram_tensor("xk", (C_IN, KB), f16, kind="ExternalInput").ap()
    wq = nc.dram_tensor("wq", (C_IN, F), f16, kind="ExternalInput").ap()
    wk = nc.dram_tensor("wk", (C_IN, F), f16, kind="ExternalInput").ap()
    wv = nc.dram_tensor("wv", (C_IN, F), f16, kind="ExternalInput").ap()
    wo2 = nc.dram_tensor("wo2", (P, NG, C_IN), f16, kind="ExternalInput").ap()
    ebT = nc.dram_tensor("ebT", (HG, KB, S), f16, kind="ExternalInput").ap()
    outT = nc.dram_tensor("outT", (C_IN, S), f16, kind="ExternalOutput").ap()

    with tile.TileContext(nc) as tc:
        with ExitStack() as ctx:
            persist = ctx.enter_context(tc.tile_pool(name="persist", bufs=1))
            v_sb = persist.tile([P, ku, HG * VW], f16)
            qT_sb = persist.tile([P, NG, S], f16)
            kT_sb = persist.tile([P, NG, KB], f16)
            wo2_sb = persist.tile([P, NG, C_IN], f16)
            oT2_sb = persist.tile([P, NG, S], f16)
            nc.scalar.dma_start(wo2_sb[:], wo2)
            # ones columns for the softmax denominator (k-padding is killed
            # by eb == 0, so the ones column itself is unmasked)
            v_view = v_sb.rearrange("p t (h c) -> p t h c", c=VW)
            nc.vector.memset(v_view[:, :, :, C:C + 1], 1.0)

            # ---- phase A input staging ----
            pa = ctx.enter_context(tc.tile_pool(name="phaseA", bufs=1))
            xq_sb = pa.tile([P, KT, S], f16)
            xk_sb = pa.tile([P, KT, KB], f16)
            wq_sb = pa.tile([P, KT, F], f16)
            wk_sb = pa.tile([P, KT, F], f16)
            wv_sb = pa.tile([P, KT, F], f16)
            # one batched DMA per tensor (per-tile DMAs pay ~0.6us of issue
            # cost each); issue order matters: qkproj(0) consumes wq/xq.
            nc.sync.dma_start(wq_sb[:], wq.rearrange("(t p) f -> p t f", p=P))
            nc.sync.dma_start(xq_sb[:], xq.rearrange("(t p) s -> p t s", p=P))
            nc.sync.dma_start(wk_sb[:], wk.rearrange("(t p) f -> p t f", p=P))
            nc.sync.dma_start(xk_sb[:], xk.rearrange("(t p) s -> p t s", p=P))
            # wv goes out on the scalar engine's queue, concurrent with the
            # sync queue, so vproj isn't gated behind the whole input stream
            nc.scalar.dma_start(wv_sb[:], wv.rearrange("(t p) f -> p t f", p=P))

            # ---- interleaved q/k projections + attention ----
            attn_ctx = ctx.enter_context(ExitStack())
            ps_pool = ctx.enter_context(
                tc.tile_pool(name="ps", bufs=4, space="PSUM"))
            oap_pool = attn_ctx.enter_context(
                tc.tile_pool(name="oap", bufs=2, space="PSUM"))
            ebt_pool = attn_ctx.enter_context(tc.tile_pool(name="ebt", bufs=3))
            ptr_pool = attn_ctx.enter_context(tc.tile_pool(name="ptr", bufs=3))
            pt_pool = attn_ctx.enter_context(tc.tile_pool(name="pt", bufs=3))
            rc_pool = attn_ctx.enter_context(tc.tile_pool(name="rc", bufs=2))
            rcb_pool = attn_ctx.enter_context(tc.tile_pool(name="rcb", bufs=2))
            tmpo_pool = attn_ctx.enter_context(
                tc.tile_pool(name="tmpo", bufs=2))

            def qkproj(mt):
                for w_sb, dst, nfree in ((wq_sb, qT_sb, S), (wk_sb, kT_sb, KB)):
                    for lo in range(0, nfree, 512):
                        hi = min(lo + 512, nfree)
                        ps = ps_pool.tile([P, 512], f32, name="ps")
                        for kt in range(KT):
                            nc.tensor.matmul(
                                ps[:, 0:hi - lo],
                                w_sb[:, kt, mt * P:(mt + 1) * P],
                                (xq_sb if dst is qT_sb else xk_sb)[
                                    :, kt, lo:hi],
                                start=(kt == 0), stop=(kt == KT - 1))
                        nc.scalar.copy(dst[:, mt, lo:hi], ps[:, 0:hi - lo])

            def vproj(mt):
                psv = ps_pool.tile([P, F], f32, name="ps")
                for kt in range(KT):
                    nc.tensor.matmul(
                        psv[:],
                        xk_sb[:, kt, mt * P:(mt + 1) * P],
                        wv_sb[:, kt, :],
                        start=(kt == 0), stop=(kt == KT - 1))
                nc.vector.tensor_copy(
                    v_view[:, mt, :, 0:C],
                    psv.rearrange("p (h c) -> p h c", c=C))

            def attn(h):
                g = h // 2
                po = (h % 2) * C
                kT_h = kT_sb[po:po + C, g, :]
                qT_h = qT_sb[po:po + C, g, :]
                oaps = oap_pool.tile([VW, S], f32, name="oaug")
                for kt in range(ku):
                    ebt = ebt_pool.tile([P, S], f16, name="ebt")
                    nc.sync.dma_start(ebt[:], ebT[h, kt * P:(kt + 1) * P, :])
                    for nh in range(2):
                        ps_s = ps_pool.tile([P, 512], f32, name="ps")
                        nc.tensor.matmul(
                            ps_s[:],
                            kT_h[:, kt * P:(kt + 1) * P],
                            qT_h[:, nh * 512:(nh + 1) * 512],
                            start=True, stop=True)
                        ptr = ptr_pool.tile([P, 512], f16, name="ptr")
                        nc.scalar.activation(ptr[:], ps_s[:],
                                             mybir.ActivationFunctionType.Exp)
                        pt = pt_pool.tile([P, 512], f16, name="pt")
                        nc.vector.tensor_mul(pt[:], ptr[:],
                                             ebt[:, nh * 512:(nh + 1) * 512])
                        nc.tensor.matmul(
                            oaps[:, nh * 512:(nh + 1) * 512],
                            v_sb[:, kt, h * VW:(h + 1) * VW],
                            pt[:],
                            start=(kt == 0), stop=(kt == ku - 1))
                # denominator row C of oaps -> partition 0 via SBUF DMA hop,
                # recip, gpsimd broadcast
                rc = rc_pool.tile([P, S], f32, name="rc")
                nc.vector.tensor_copy(rc[C:C + 1, :], oaps[C:C + 1, :])
                rc0 = rc_pool.tile([1, S], f32, name="rc0", tag="rc0")
                nc.sync.dma_start(rc0[:], rc[C:C + 1, :])
                rcv = rc_pool.tile([1, S], f32, name="rcv", tag="rcv")
                nc.vector.reciprocal_approx_fast(rcv[:], rc0[:])
                rcb = rcb_pool.tile([C, S], f32, name="rcb")
                nc.gpsimd.partition_broadcast(rcb[:], rcv[:])
                if h % 2 == 0:
                    nc.vector.tensor_mul(oT2_sb[0:C, g, :], oaps[0:C, :],
                                         rcb[:])
                else:
                    tmp = tmpo_pool.tile([C, S], f16, name="tmpo")
                    nc.vector.tensor_mul(tmp[:], oaps[0:C, :], rcb[:])
                    nc.sync.dma_start(oT2_sb[C:P, g, :], tmp[:])

            qkproj(0)
            for mt in range(ku):
                vproj(mt)
            attn(0)
            qkproj(1)
            attn(1)
            qkproj(2)
            attn(2)
            attn(3)
            qkproj(3)
            for h in range(4, HG):
                attn(h)
            attn_ctx.close()

            # ---- output projection: outT = wo2^T @ oT2 (pair-stacked) ----
            with tc.tile_pool(name="outsb", bufs=4) as out_pool:
                for ct in range(KT):
                    for nh in range(2):
                        pso = ps_pool.tile([P, 512], f32, name="ps")
                        for g in range(NG):
                            nc.tensor.matmul(
                                pso[:],
                                wo2_sb[:, g, ct * P:(ct + 1) * P],
                                oT2_sb[:, g, nh * 512:(nh + 1) * 512],
                                start=(g == 0), stop=(g == NG - 1))
                        osb = out_pool.tile([P, 512], f16, name="osb")
                        if (ct * 2 + nh) % 2 == 0:
                            nc.scalar.copy(osb[:], pso[:])
                        else:
                            nc.vector.tensor_copy(osb[:], pso[:])
                        nc.sync.dma_start(
                            outT[ct * P:(ct + 1) * P,
                                 nh * 512:(nh + 1) * 512],
                            osb[:])

    nc.compile()
    return nc


def make_in_maps(x, bias, attention_mask, Wq, Wk, Wv, Wo):
    x = np.asarray(x)
    bias = np.asarray(bias)
    mask = np.asarray(attention_mask)
    scale = 1.0 / math.sqrt(C)
    wq16 = (np.asarray(Wq) * scale).astype(np.float16)
    wk16 = np.asarray(Wk).astype(np.float16)
    wv16 = np.asarray(Wv).astype(np.float16)
    wo = np.asarray(Wo)

    counts = mask.sum(axis=1)
    ku = max(1, int(math.ceil(counts.max() / P)))
    KB = ku * P

    # per-batch permutation: unmasked k-positions first
    idxs = []
    for b in range(B):
        order = np.argsort(~mask[b].astype(bool), kind="stable")
        idxs.append(order[:KB])

    in_maps = []
    for c in range(N_CORES):
        b, hg = c // 2, c % 2
        fs = slice(hg * F, (hg + 1) * F)
        idx = idxs[b]
        xT = x[b].T.astype(np.float16)  # (c_in, S)
        # eb = exp(bias) * mask, permuted/truncated on k, transposed to (k,q)
        eb = np.exp(bias[b, hg * HG:(hg + 1) * HG][:, :, idx])
        eb *= mask[b][idx].astype(np.float32)[None, None, :]
        ebT = np.ascontiguousarray(eb.transpose(0, 2, 1)).astype(np.float16)
        wo_c = wo[fs].astype(np.float16)  # (512, c_in)
        wo2 = np.ascontiguousarray(
            wo_c.reshape(NG, 2, C, C_IN).transpose(1, 2, 0, 3)
        ).reshape(P, NG, C_IN)
        in_maps.append({
            "xq": np.ascontiguousarray(xT),
            "xk": np.ascontiguousarray(xT[:, idx]),
            "wq": np.ascontiguousarray(wq16[:, fs]),
            "wk": np.ascontiguousarray(wk16[:, fs]),
            "wv": np.ascontiguousarray(wv16[:, fs]),
            "wo2": wo2,
            "ebT": ebT,
        })
    return in_maps, ku


_NC_CACHE = {}


def get_program(ku=5):
    if ku not in _NC_CACHE:
        _NC_CACHE[ku] = build_program(ku)
    return _NC_CACHE[ku]


def run(in_maps, ku, trace=False, **kw):
    nc = get_program(ku)
    return run_bass_kernel_spmd(nc, in_maps, core_ids=list(range(N_CORES)),
                                trace=trace, **kw)


def kernel(x, bias, attention_mask, Wq, Wk, Wv, Wo, bo):
    in_maps, ku = make_in_maps(x, bias, attention_mask, Wq, Wk, Wv, Wo)
    res = run(in_maps, ku)
    out = np.empty((B, S, C_IN), dtype=np.float32)
    bo32 = np.asarray(bo).astype(np.float32)
    for b in range(B):
        acc = (res.results[2 * b]["outT"].astype(np.float32)
               + res.results[2 * b + 1]["outT"].astype(np.float32))
        out[b] = acc.T + bo32
    return out
